# revision 13
# baseline (speedup 1.0000x reference)
"""ConvDecoder Bass kernel for Trainium2, SPMD over 8 NeuronCores.

Math (per batch element b, one per core):
    r_conv = Conv1d(r, conv_w, SAME) + conv_b            # (C, N_IN)
    d[n,m] = (xc[n] - xt[m])^2                           # (N_IN, N_OUT)
    wt_c   = exp(-0.5 * d / exp(sigma_c)^2)
    z[m,c] = sum_n r_conv[c,n] * wt_c[n,m]
    out    = z @ lin_w.T + lin_b                         # (N_OUT, OUT_C)

Kernel structure per core:
  - Conv1d as an im2col matmul: 5 shifted DMA copies of r (+ a ones row for
    the bias) stacked on partitions -> (81, 512); matmul with repacked
    weights (81, 16) yields the conv output directly in (n, c) layout,
    which is exactly the lhsT the RBF-reduction matmul needs.
  - Channels sharing a length scale share one RBF map. Channels are
    grouped host-side (permuting conv_w's out axis / lin_w's in axis);
    with the uniform init sigma there is a single group, so one E map.
  - sqrt(a)*xc[n] - sqrt(a)*xt[m] (a = 0.5/scale^2) is a K=2 matmul on the
    tensor engine into PSUM; the vector engine squares it; the scalar
    engine exponentiates -> E tile (128, 512). Three engines pipeline
    across the 8 (n-tile, m-half) chunks.
  - z[c,m] accumulates over the 4 n-tiles in PSUM; the final linear is a
    (17,128)^T @ (17,32) matmul per m-tile with a ones row folding lin_b.
"""

import numpy as np

import concourse.bass as bass
import concourse.mybir as mybir
from concourse.tile import TileContext
from concourse.bass_utils import run_bass_kernel_spmd

F32 = mybir.dt.float32

B, N_IN, N_OUT, C, OUT_C, KW = 8, 512, 1024, 16, 32, 5
N_CORES = 8
NT = N_IN // 128   # n tiles (4)
MH = N_OUT // 512  # m halves (2)
MT = 512 // 128    # m tiles per half (4)


# --- walrus workaround -----------------------------------------------------
# This container's walrus accepts at most ONE semaphore wait per TPB
# instruction, but Tile's scheduler attaches several (joins + tail drain).
# Hoist all but the last wait of each instruction onto fresh wait-only
# EventSemaphore instructions inserted right before it on the same engine.
_ws_ctr = [0]


def _split_multi_waits(nc):
    for fn in nc.m.functions:
        for blk in fn.blocks:
            insts = blk.instructions
            if not any(
                ins.sync_info and len(ins.sync_info.on_wait) > 1 for ins in insts
            ):
                continue
            out = []
            for ins in insts:
                si = ins.sync_info
                waits = list(si.on_wait) if si else []
                if len(waits) > 1:
                    for w in waits[:-1]:
                        _ws_ctr[0] += 1
                        ev = mybir.InstEventSemaphore(
                            name=f"waitsplit_{_ws_ctr[0]}", ins=[], outs=[]
                        )
                        ev.engine = ins.engine
                        ev.sync_info = mybir.SyncInfo(on_wait=[w], on_update=[])
                        nc.register_instruction(ev)
                        out.append(ev)
                    ins.sync_info = mybir.SyncInfo(
                        on_wait=[waits[-1]], on_update=list(si.on_update)
                    )
                out.append(ins)
            insts[:] = out


# --- kernel build ----------------------------------------------------------
def _build(groups):
    """groups: tuple of (c0, c1, sqrt_a) with contiguous channel ranges."""
    nc = bass.Bass()
    r_in = nc.dram_tensor("r", [C, N_IN], F32, kind="ExternalInput")
    xc_in = nc.dram_tensor("xc", [1, N_IN], F32, kind="ExternalInput")
    xt_in = nc.dram_tensor("xt", [1, N_OUT], F32, kind="ExternalInput")
    wconv = nc.dram_tensor("w_aug", [C * KW + 1, C], F32, kind="ExternalInput")
    wlin = nc.dram_tensor("lin_w_t", [C, OUT_C], F32, kind="ExternalInput")
    blin = nc.dram_tensor("lin_b", [1, OUT_C], F32, kind="ExternalInput")
    y_out = nc.dram_tensor("y", [N_OUT, OUT_C], F32, kind="ExternalOutput")

    Exp = mybir.ActivationFunctionType.Exp

    with TileContext(nc) as tc:
        with (
            tc.tile_pool(name="const", bufs=1) as cpool,
            tc.tile_pool(name="work", bufs=1) as wpool,
            tc.tile_pool(name="psum", bufs=1, space="PSUM") as ppool,
        ):
            # ---- constants / operand rows ----
            wa = cpool.tile([C * KW + 1, C], F32)
            nc.sync.dma_start(out=wa[:], in_=wconv[:])
            wl = cpool.tile([C, OUT_C], F32)
            nc.sync.dma_start(out=wl[:], in_=wlin[:])
            bl = cpool.tile([1, OUT_C], F32)
            nc.sync.dma_start(out=bl[:], in_=blin[:])

            xc_row = cpool.tile([1, N_IN], F32)
            nc.sync.dma_start(out=xc_row[:], in_=xc_in[:])
            diffR = cpool.tile([2, N_OUT], F32)
            nc.vector.memset(diffR[0:1, :], 1.0)
            nc.sync.dma_start(out=diffR[1:2, :], in_=xt_in[:])

            ones_row = cpool.tile([1, 512], F32)
            nc.vector.memset(ones_row[:], 1.0)

            diffLs = []
            for gi, (c0, c1, sa) in enumerate(groups):
                # full-tile memset to -sqrt(a); row 0 then overwritten with
                # sqrt(a)*xc (compute ops must start at 32-aligned partitions,
                # so per-row memsets at partition 1 are not allowed)
                dl = cpool.tile([2, N_IN], F32, name=f"diffL{gi}")
                nc.vector.memset(dl[:, :], -float(sa))
                nc.vector.tensor_scalar_mul(dl[0:1, :], xc_row[:], float(sa))
                diffLs.append(dl)

            # ---- conv im2col stack: row 0 = ones (bias), rows 1+16k+ci ----
            stack = cpool.tile([C * KW + 1, N_IN], F32)
            nc.vector.memset(stack[:, :], 0.0)
            pad = KW // 2
            for k in range(KW):
                lo = max(0, pad - k)
                hi = min(N_IN, N_IN + pad - k)
                nc.sync.dma_start(
                    out=stack[1 + C * k : 1 + C * (k + 1), lo:hi],
                    in_=r_in[:, lo + k - pad : hi + k - pad],
                )
            # ones row via DMA-exempt path is not needed: row 0 is
            # partition 0, so a plain memset is legal
            nc.vector.memset(stack[0:1, :], 1.0)

            # ---- conv matmuls: (81,128)^T @ (81,16) -> (128,16) per n-tile ----
            r_t = []
            for t in range(NT):
                cps = ppool.tile([128, C], F32, tag="smallps", bufs=2,
                                 name=f"cps{t}")
                nc.tensor.matmul(
                    cps[:],
                    lhsT=stack[:, t * 128 : (t + 1) * 128],
                    rhs=wa[:],
                    start=True,
                    stop=True,
                )
                rsb = cpool.tile([128, C], F32, name=f"rsb{t}")
                nc.vector.tensor_copy(out=rsb[:], in_=cps[:])
                r_t.append(rsb)

            # ---- main pipeline over m-halves / groups / n-tiles ----
            for mh in range(MH):
                z_sb = wpool.tile([C, 512], F32, tag="zsb", bufs=2, name=f"z{mh}")
                for gi, (c0, c1, sa) in enumerate(groups):
                    gsz = c1 - c0
                    zps = ppool.tile([gsz, 512], F32, tag="zps", bufs=2,
                                     name=f"zps{mh}_{gi}")
                    for k in range(NT):
                        dps = ppool.tile([128, 512], F32, tag="dps", bufs=3,
                                         name=f"dps{mh}_{gi}_{k}")
                        nc.tensor.matmul(
                            dps[:],
                            lhsT=diffLs[gi][:, k * 128 : (k + 1) * 128],
                            rhs=diffR[:, mh * 512 : (mh + 1) * 512],
                            start=True,
                            stop=True,
                        )
                        dcp = wpool.tile([128, 512], F32, tag="dcp", bufs=3,
                                         name=f"dcp{mh}_{gi}_{k}")
                        nc.vector.tensor_copy(out=dcp[:], in_=dps[:])
                        dsq = wpool.tile([128, 512], F32, tag="dsq", bufs=3,
                                         name=f"dsq{mh}_{gi}_{k}")
                        nc.vector.tensor_mul(out=dsq[:], in0=dcp[:], in1=dcp[:])
                        esb = wpool.tile([128, 512], F32, tag="esb", bufs=3,
                                         name=f"e{mh}_{gi}_{k}")
                        nc.scalar.activation(esb[:], dsq[:], Exp, scale=-1.0)
                        nc.tensor.matmul(
                            zps[:],
                            lhsT=r_t[k][:, c0:c1],
                            rhs=esb[:],
                            start=(k == 0),
                            stop=(k == NT - 1),
                        )
                    if c0 % 32 == 0:
                        nc.vector.tensor_copy(out=z_sb[c0:c1, :], in_=zps[:])
                    else:
                        # compute engines need 32-aligned partition starts;
                        # DMA is exempt
                        nc.sync.dma_start(out=z_sb[c0:c1, :], in_=zps[:])

                for mt in range(MT):
                    ops = ppool.tile([128, OUT_C], F32, tag="smallps", bufs=2,
                                     name=f"ops{mh}_{mt}")
                    nc.tensor.matmul(
                        ops[:],
                        lhsT=z_sb[:, mt * 128 : (mt + 1) * 128],
                        rhs=wl[:],
                        start=True,
                        stop=False,
                    )
                    nc.tensor.matmul(
                        ops[:],
                        lhsT=ones_row[:, mt * 128 : (mt + 1) * 128],
                        rhs=bl[:],
                        start=False,
                        stop=True,
                    )
                    osb = wpool.tile([128, OUT_C], F32, tag="osb", bufs=3,
                                     name=f"o{mh}_{mt}")
                    nc.vector.tensor_copy(out=osb[:], in_=ops[:])
                    m0 = mh * 512 + mt * 128
                    nc.sync.dma_start(out=y_out[m0 : m0 + 128, :], in_=osb[:])

    _split_multi_waits(nc)
    return nc


_cache = {}


def _get_nc(groups):
    key = tuple((c0, c1, np.float32(sa).tobytes()) for c0, c1, sa in groups)
    if key not in _cache:
        _cache[key] = _build(groups)
    return _cache[key]


def _prepare(r, x_context, y_context, x_target, conv_w, conv_b, sigma, lin_w,
             lin_b):
    r = np.asarray(r, np.float32)
    x_context = np.asarray(x_context, np.float32)
    x_target = np.asarray(x_target, np.float32)
    conv_w = np.asarray(conv_w, np.float32)
    conv_b = np.asarray(conv_b, np.float32)
    sigma = np.asarray(sigma, np.float32)
    lin_w = np.asarray(lin_w, np.float32)
    lin_b = np.asarray(lin_b, np.float32)

    # Channels sharing a length scale share one RBF map: sort channels by a,
    # group runs of equal values (uniform init sigma -> a single group).
    scales = np.exp(sigma.astype(np.float64))
    a = 0.5 / scales**2
    perm = np.argsort(a, kind="stable")
    a_s = a[perm]
    groups = []
    c0 = 0
    for c in range(1, C + 1):
        if c == C or a_s[c] != a_s[c0]:
            groups.append((c0, c, float(np.sqrt(a_s[c0]))))
            c0 = c
    groups = tuple(groups)

    # Repack weights (channel-permuted; conv bias row first to stay
    # partition-0-aligned with the im2col ones row).
    w_aug = np.concatenate(
        [conv_b[None, :], conv_w.transpose(2, 1, 0).reshape(C * KW, C)], axis=0
    )[:, perm]
    w_aug = np.ascontiguousarray(w_aug, np.float32)
    lin_w_t = np.ascontiguousarray(lin_w.T[perm], np.float32)
    lin_b_row = np.ascontiguousarray(lin_b[None, :], np.float32)

    in_maps = [
        {
            "r": np.ascontiguousarray(r[b]),
            "xc": np.ascontiguousarray(x_context[b].reshape(1, N_IN)),
            "xt": np.ascontiguousarray(x_target[b].reshape(1, N_OUT)),
            "w_aug": w_aug,
            "lin_w_t": lin_w_t,
            "lin_b": lin_b_row,
        }
        for b in range(B)
    ]
    return groups, in_maps


def kernel(**inputs):
    groups, in_maps = _prepare(**inputs)
    nc = _get_nc(groups)
    res = run_bass_kernel_spmd(nc, in_maps, list(range(N_CORES)))
    return np.stack([res.results[b]["y"] for b in range(B)], axis=0)
